# revision 48
# baseline (speedup 1.0000x reference)
"""Trainium2 Bass kernel for a 6-layer pre-LN transformer encoder (nn_Encoder).

Distribution: tokens sharded 8 ways (core c -> batch c//4, seq chunk c%4 of 512
tokens).  Per layer, each core computes K/V projections for its own tokens and
AllGathers them (bf16) within its batch group of 4 cores, then computes
attention for its 512 query tokens over the full 2048-key sequence.

Pipeline (per layer, emission order == desired schedule):
  LN1_A -> K/V_A -> AG0(tokens A)          # AG0 in flight...
  FFN(l-1)_B (+residual)                   # ...hidden under prev-layer FFN B
  LN1_B -> K/V_B -> AG1(tokens B) -> Q
  attention: pairs01 partA (spill av/z to SBUF) -> pairs23 partA+B ->
             finalize23 -> pairs01 partB -> finalize01   # AG1 hides under partA
  O-proj + residual -> LN2_A -> FFN_A -> LN2_B   [FFN_B deferred to next iter]

The residual stream is kept transposed (x.T: [D, tok]), split in two token
halves xTA/xTB so the halves pipeline without false dependencies.
Softmax: ep = exp(scores + mask_bias) on ACT; z via ones-matmul; the
normalizer 1/z = Exp(-Ln(z)) on ACT (DVE reciprocal on [1,T] is ~3.3us).
All ACT functions (Exp, Ln) resolve to the single natural_log_exp_and_others
table set (see _patch_act_tables) so no ACT_TABLE_LOAD thrash.
"""

import math

import numpy as np
import ml_dtypes

import concourse.bass as bass
import concourse.mybir as mybir
import concourse.tile as tile
from concourse import bacc
from concourse.bass_utils import run_bass_kernel_spmd

# ---------------------------------------------------------------------------
# Force Exp and Ln into one ACT table set. The default placement pass maps
# Exp -> exp_and_others and Ln -> natural_log, thrashing ACT_TABLE_LOADs
# (~2.7us each) on every LN <-> attention transition. Hiding the
# single-function sets makes both resolve to natural_log_exp_and_others.
# Indices into act_info.json are preserved (names keep their positions).
import concourse.hw_specs as _hw_specs
import concourse.bacc as _bacc_mod

_orig_get_tables = _hw_specs.get_activation_tables


def _patched_tables(arch):
    t = _orig_get_tables(arch)
    hide = ("exp_and_others", "natural_log", "exp_and_friends")
    return {name: (set() if name in hide else fns) for name, fns in t.items()}


_bacc_mod.get_activation_tables = _patched_tables

F32 = mybir.dt.float32
BF16 = mybir.dt.bfloat16
I32 = mybir.dt.int32
AF = mybir.ActivationFunctionType
OP = mybir.AluOpType

VOCAB, D, H, L, DFF, PAD = 32000, 512, 8, 6, 2048, 0
B, S = 2, 2048
DK = D // H          # 64
P = 128
T = 512              # tokens per core
NJ = D // P          # 4   D-chunks
NM = DFF // P        # 16  DFF-chunks
NC = S // P          # 16  key chunks per batch row
HALF = 256           # tokens per half
HC = HALF // P       # 2   token-chunks per half
NCORES = 8
GROUP = 4            # cores per batch group
EPS = 1e-5
SQRT_D = math.sqrt(D)
NEG = -80.0          # mask bias: exp(s-80) is zero for practical purposes
KVE = P * NJ * HALF  # elements per AG section (K: [P,NJ,HALF], V: [P,HC,D])

REPLICA_GROUPS = [[0, 1, 2, 3], [4, 5, 6, 7]]


def build_kernel(use_b1, use_b2, use_ln1, use_ln2, use_fn):
    nc = bacc.Bacc("TRN2", target_bir_lowering=False, debug=False,
                   num_devices=NCORES)

    # register EPS as a const AP so activation(bias=EPS) works
    _eps_t = nc.alloc_sbuf_tensor("const-float32-eps", [P, 1], F32)
    nc.gpsimd.memset(_eps_t.ap(), EPS)
    nc.const_aps.aps[(F32, EPS)] = _eps_t.ap()

    # ---------------- parameters ----------------
    tok = nc.declare_dram_parameter("tok", [P, NJ], I32, isOutput=False)
    emb = nc.declare_dram_parameter("emb", [VOCAB, D], F32, isOutput=False)
    pet = nc.declare_dram_parameter("pet", [P, NJ, T], F32, isOutput=False)
    wq = nc.declare_dram_parameter("wq", [L, P, NJ, D], BF16, isOutput=False)
    wk = nc.declare_dram_parameter("wk", [L, P, NJ, D], BF16, isOutput=False)
    wv = nc.declare_dram_parameter("wv", [L, P, NJ, D], BF16, isOutput=False)
    wo = nc.declare_dram_parameter("wo", [L, P, NJ, D], BF16, isOutput=False)
    w1 = nc.declare_dram_parameter("w1", [L, P, NJ, DFF], BF16, isOutput=False)
    w2 = nc.declare_dram_parameter("w2", [L, P, NM, D], BF16, isOutput=False)
    b1t = nc.declare_dram_parameter("b1t", [L, P, NM], F32, isOutput=False)
    b2t = nc.declare_dram_parameter("b2t", [L, P, NJ], F32, isOutput=False)
    lnsb = nc.declare_dram_parameter("lnsb", [P, 2 * L + 1, 2, NJ], F32,
                                     isOutput=False)
    kmaskn = nc.declare_dram_parameter("kmaskn", [P, NC], F32, isOutput=False)
    ones_bf = nc.declare_dram_parameter("ones_bf", [P, 1], BF16, isOutput=False)
    oneD = nc.declare_dram_parameter("oneD", [P, 1], F32, isOutput=False)
    ones_row = nc.declare_dram_parameter("ones_row", [33, P], F32,
                                         isOutput=False)
    ident = nc.declare_dram_parameter("ident", [P, P], F32, isOutput=False)
    y = nc.declare_dram_parameter("y", [T, D], F32, isOutput=True)

    use_lnp = use_ln1 or use_ln2 or use_fn

    with tile.TileContext(nc) as tc:
        with (
            tc.tile_pool(name="wpool", bufs=2) as wpool,
            tc.tile_pool(name="work", bufs=1) as work,
            tc.tile_pool(name="small", bufs=2) as small,
            tc.tile_pool(name="kvp", bufs=2) as kvp,
            tc.tile_pool(name="expp", bufs=4) as expp,
            tc.tile_pool(name="const", bufs=1) as constp,
            tc.tile_pool(name="ps_sc", bufs=3, space="PSUM") as ps_sc,
            tc.tile_pool(name="ps_acc", bufs=2, space="PSUM") as ps_acc,
            tc.tile_pool(name="dram", bufs=2, space="DRAM") as dram,
        ):
            # ---------------- constants / persistent state ----------------
            ones_bf_sb = constp.tile([P, 1], BF16)
            nc.sync.dma_start(ones_bf_sb[:], ones_bf[:])
            oneD_sb = constp.tile([P, 1], F32)
            nc.sync.dma_start(oneD_sb[:], oneD[:])
            ones_row_sb = constp.tile([33, P], F32)
            nc.sync.dma_start(ones_row_sb[:], ones_row[:])
            ident_sb = constp.tile([P, P], F32)
            nc.sync.dma_start(ident_sb[:], ident[:])
            # pet is only read during embedding; park it in the h1T slot
            # (first FFN h1T alloc comes after all embedding reads)
            pet_sb = work.tile([P, NJ, T], F32, tag="h10", name="pet")
            nc.sync.dma_start(pet_sb[:], pet[:])
            kmask_sb = constp.tile([P, NC], F32)
            nc.sync.dma_start(kmask_sb[:], kmaskn[:])
            tok_sb = constp.tile([P, NJ], I32)
            nc.sync.dma_start(tok_sb[:], tok[:])
            lnp_sb = None
            if use_lnp:
                lnp_sb = constp.tile([P, 2 * L + 1, 2, NJ], F32, name="lnp")
                nc.sync.dma_start(lnp_sb[:], lnsb[:])

            # residual stream x.T, split in token halves
            xTh = (constp.tile([P, NJ, HALF], F32, name="xTA"),
                   constp.tile([P, NJ, HALF], F32, name="xTB"))

            # ---------------- helpers ----------------
            # LayerNorm over D (partition axis) of xh [P,NJ,HALF], split so
            # the stats matmuls can interleave with the producer of xh:
            #   ln_begin -> (ln_stats per j-chunk, as chunks become ready)
            #   -> ln_finish
            def ln_begin(h):
                st = ps_acc.tile([P, HALF], F32, tag="acc", name="st")
                sq = work.tile([P, NJ, HALF], F32, tag=f"lnt{h}", name="sq")
                return (st, sq)

            def ln_stats(ctx, xh, j, first=None, last=None):
                st, sq = ctx
                if first is None:
                    first = j == 0
                if last is None:
                    last = j == NJ - 1
                nc.vector.tensor_tensor(sq[:, j, :], xh[:, j, :], xh[:, j, :],
                                        OP.mult)
                nc.tensor.matmul(st[0:1, :], lhsT=oneD_sb[:], rhs=xh[:, j, :],
                                 start=first, stop=last,
                                 tile_position=(0, 0), skip_group_check=True)
                nc.tensor.matmul(st[32:33, :], lhsT=oneD_sb[:], rhs=sq[:, j, :],
                                 start=first, stop=last,
                                 tile_position=(0, 32), skip_group_check=True)

            def ln_finish(ctx, dst, xh, h, param_idx, use_params):
                st, sq = ctx
                mu = small.tile([1, HALF], F32, tag="mu")
                nc.vector.tensor_copy(mu[:], st[0:1, :])
                var = small.tile([1, HALF], F32, tag="var")
                nc.vector.tensor_tensor(var[:], mu[:], mu[:], OP.mult)
                nc.vector.tensor_tensor(var[:], st[32:33, :], var[:],
                                        OP.subtract)
                lnv = small.tile([1, HALF], F32, tag="lnv")
                nc.scalar.activation(lnv[:], var[:], AF.Ln, bias=EPS)
                rinv = small.tile([1, HALF], F32, tag="rinv")
                nc.scalar.activation(rinv[:], lnv[:], AF.Exp, scale=-0.5)
                bc = work.tile([P, 2, HALF], F32, tag="lnbc", name="lnbc")
                nc.gpsimd.partition_broadcast(bc[:, 0, :], mu[:])
                nc.gpsimd.partition_broadcast(bc[:, 1, :], rinv[:])
                t1 = work.tile([P, NJ, HALF], F32, tag=f"lnt{h}", name="lnt")
                nc.vector.tensor_tensor(
                    t1[:], xh[:], bc[:, 0, :][:, None, :].to_broadcast(
                        [P, NJ, HALF]), OP.subtract)
                nc.vector.tensor_tensor(
                    dst[:], t1[:], bc[:, 1, :][:, None, :].to_broadcast(
                        [P, NJ, HALF]), OP.mult)
                if use_params:
                    for j in range(NJ):
                        nc.vector.tensor_scalar(
                            dst[:, j, :], dst[:, j, :],
                            lnp_sb[:, param_idx, 0, j:j + 1],
                            lnp_sb[:, param_idx, 1, j:j + 1],
                            OP.mult, OP.add)



            def load_qkvo(l):
                t = {}
                for nm, src in (("wq", wq), ("wk", wk), ("wv", wv), ("wo", wo)):
                    t[nm] = wpool.tile([P, NJ, D], BF16, tag=nm, name=nm)
                    nc.sync.dma_start(t[nm][:], src[l])
                return t

            def load_ffn(t, l):
                t["w1"] = wpool.tile([P, NJ, DFF], BF16, tag="w1", bufs=1,
                                     name="w1")
                nc.sync.dma_start(t["w1"][:], w1[l])
                t["w2"] = wpool.tile([P, NM, D], BF16, tag="w2", bufs=1,
                                     name="w2")
                nc.sync.dma_start(t["w2"][:], w2[l])
                if use_b1:
                    t["b1"] = wpool.tile([P, NM], F32, tag="b1", bufs=1,
                                         name="b1")
                    nc.sync.dma_start(t["b1"][:], b1t[l])
                if use_b2:
                    t["b2"] = wpool.tile([P, NJ], F32, tag="b2", bufs=1,
                                         name="b2")
                    nc.sync.dma_start(t["b2"][:], b2t[l])

            def emb_half(h):
                """Embedding gather + transpose for token half h (0:A, 1:B)."""
                xH = xTh[h]
                for jj in range(HC):
                    j = HC * h + jj
                    ex = small.tile([P, D], F32, tag="embx", bufs=1)
                    nc.gpsimd.indirect_dma_start(
                        out=ex[:],
                        out_offset=None,
                        in_=emb[:, :],
                        in_offset=bass.IndirectOffsetOnAxis(
                            ap=tok_sb[:, j:j + 1], axis=0),
                    )
                    for dj in range(NJ):
                        pst = ps_sc.tile([P, P], F32, tag="s", name="etr")
                        nc.tensor.transpose(pst[:], ex[:, dj * P:(dj + 1) * P],
                                            ident_sb[:])
                        sl = xH[:, dj, jj * P:(jj + 1) * P]
                        nc.vector.tensor_scalar(sl, pst[:], SQRT_D, None,
                                                OP.mult)
                        nc.vector.tensor_add(sl, sl,
                                             pet_sb[:, dj, j * P:(j + 1) * P])

            def kv_half(w_sb, hT, h):
                """K/V projections for token half h, staged to DRAM
                per-chunk so the AllGather staging overlaps the matmuls."""
                kv_h = dram.tile([2, KVE], BF16, tag=f"kv{h}", name=f"kv{h}")
                kview = kv_h[0].rearrange("(m p t) -> p m t", p=P, m=NJ)
                vview = kv_h[1].rearrange("(m p d) -> p m d", p=P, m=HC)
                kTh = work.tile([P, NJ, HALF], BF16, tag=f"kT{h}", name="kT")
                for m in range(NJ):
                    ps = ps_sc.tile([P, HALF], F32, tag="s", name="kmm")
                    for j in range(NJ):
                        nc.tensor.matmul(
                            ps[:], lhsT=w_sb["wk"][:, j, m * P:(m + 1) * P],
                            rhs=hT[:, j, :],
                            start=(j == 0), stop=(j == NJ - 1),
                            tile_position=(0, 0))
                    nc.vector.tensor_copy(kTh[:, m, :], ps[:])
                    nc.sync.dma_start(kview[:, m, :], kTh[:, m, :])
                vlh = work.tile([P, HC, D], BF16, tag=f"vl{h}", name="vl")
                for tc_ in range(HC):
                    ps = ps_sc.tile([P, D], F32, tag="s", name="vmm")
                    for j in range(NJ):
                        nc.tensor.matmul(
                            ps[:], lhsT=hT[:, j, tc_ * P:(tc_ + 1) * P],
                            rhs=w_sb["wv"][:, j, :],
                            start=(j == 0), stop=(j == NJ - 1),
                            tile_position=(0, 0))
                    nc.vector.tensor_copy(vlh[:, tc_, :], ps[:])
                    nc.sync.dma_start(vview[:, tc_, :], vlh[:, tc_, :])
                return kv_h

            def launch_ag(kv_h, h):
                ag_h = dram.tile([GROUP, 2, KVE], BF16, tag=f"ag{h}",
                                 name=f"ag{h}")
                nc.gpsimd.collective_compute(
                    "AllGather", OP.bypass, replica_groups=REPLICA_GROUPS,
                    ins=[kv_h[:].opt()], outs=[ag_h[:].opt()],
                )
                return ag_h

            # ---------------- start: embedding A, defer B ----------------
            w_sb = load_qkvo(0)
            load_ffn(w_sb, 0)
            emb_half(0)
            ln1a_ctx = ln_begin(0)
            for j in range(NJ):
                ln_stats(ln1a_ctx, xTh[0], j)

            def emb_tail():
                emb_half(1)
                ctx = ln_begin(1)
                for j in range(NJ):
                    ln_stats(ctx, xTh[1], j)
                return ctx

            prev_tail = [emb_tail]

            for l in range(L):
                # ---- LN1_A (stats already interleaved) + K/V_A + AG0 ----
                hTA = work.tile([P, NJ, HALF], BF16, tag="hTA", name="hTA")
                ln_finish(ln1a_ctx, hTA, xTh[0], 0, 2 * l, use_ln1)
                kvA = kv_half(w_sb, hTA, 0)
                agA = launch_ag(kvA, 0)

                # ---- previous layer's FFN B-half (or embedding B); emits
                # LN1_B stats interleaved and returns the ln ctx ----
                ln1b_ctx = prev_tail[0]()

                # ---- next layer FFN weights (after prev FFN_B emission,
                # so the single-buffered w1/w2 WAR deps are correct) ----
                if l > 0:
                    load_ffn(w_sb, l)

                # ---- LN1_B + K/V_B + AG1 + Q ----
                hTB = work.tile([P, NJ, HALF], BF16, tag="hTB", name="hTB")
                ln_finish(ln1b_ctx, hTB, xTh[1], 1, 2 * l, use_ln1)
                kvB = kv_half(w_sb, hTB, 1)
                agB = launch_ag(kvB, 1)

                qT = work.tile([P, NJ, T], BF16, tag="qT", name="qT")

                def q_chunk(m):
                    ps = ps_sc.tile([P, T], F32, tag="s", name="qmm")
                    for hh, hT in ((0, hTA), (1, hTB)):
                        for j in range(NJ):
                            nc.tensor.matmul(
                                ps[:, hh * HALF:(hh + 1) * HALF],
                                lhsT=w_sb["wq"][:, j, m * P:(m + 1) * P],
                                rhs=hT[:, j, :],
                                start=(j == 0), stop=(j == NJ - 1),
                                tile_position=(0, 0))
                    nc.vector.tensor_scalar(qT[:, m, :], ps[:],
                                            1.0 / math.sqrt(DK), None, OP.mult)

                # pair p's scores only need qT chunk m=p: compute m=0,1 now,
                # m=2,3 inside the attention window (fills ACT-bound PE slack)
                q_chunk(0)
                q_chunk(1)

                # prefetch next layer's QKVO weights (overlaps collectives)
                if l + 1 < L:
                    w_next = load_qkvo(l + 1)

                # ---- attention ----
                ags = (agA, agB)
                kts = {}
                vps = {}

                def load_part(pair, part):
                    kt = kvp.tile([P, GROUP, HALF], BF16, tag=f"kt{part}",
                                  name=f"kt{part}")
                    nc.sync.dma_start(
                        kt[:],
                        ags[part][:, 0].rearrange(
                            "r (m p t) -> m p r t", m=NJ, p=P)[pair])
                    vp = kvp.tile([P, GROUP * HC, P], BF16, tag=f"vp{part}",
                                  name=f"vp{part}")
                    for r in range(GROUP):
                        nc.sync.dma_start(
                            vp[:, r * HC:(r + 1) * HC, :],
                            ags[part][r, 1].rearrange(
                                "(m p d) -> p m d", m=HC, p=P)
                            [:, :, pair * P:(pair + 1) * P])
                    kts[(pair, part)] = kt
                    vps[(pair, part)] = vp

                attnT = work.tile([P, NJ, T], BF16, tag="attnT", name="attnT")
                avz = {}
                spl = {}

                def attn_part(pair, part, start, stop):
                    kt = kts[(pair, part)]
                    vp = vps[(pair, part)]
                    if start:
                        avz[pair] = (
                            ps_acc.tile([P, T], F32, tag="acc",
                                        name=f"av{pair}"),
                            ps_acc.tile([P, T], F32, tag="acc",
                                        name=f"z{pair}"),
                        )
                    av, z = avz[pair]
                    for ci in range(HC * GROUP):
                        gc = part * 8 + ci
                        r, tc_ = ci // HC, ci % HC
                        pss = ps_sc.tile([P, 2, T], F32, tag="s", name="pss")
                        nc.tensor.matmul(pss[:, 0, :],
                                         lhsT=kt[0:DK, r, tc_ * P:(tc_ + 1) * P],
                                         rhs=qT[0:DK, pair, :],
                                         start=True, stop=True,
                                         tile_position=(0, 0))
                        nc.tensor.matmul(pss[:, 1, :],
                                         lhsT=kt[DK:P, r, tc_ * P:(tc_ + 1) * P],
                                         rhs=qT[DK:P, pair, :],
                                         start=True, stop=True,
                                         tile_position=(64, 0))
                        ep = expp.tile([P, 2, T], BF16, tag="ep", name="ep")
                        nc.scalar.activation(ep[:], pss[:], AF.Exp,
                                             bias=kmask_sb[:, gc:gc + 1])
                        first = start and ci == 0
                        last = stop and ci == HC * GROUP - 1
                        vtile = vp[:, ci, :]
                        nc.tensor.matmul(av[0:DK, :], lhsT=vtile[:, 0:DK],
                                         rhs=ep[:, 0, :], start=first,
                                         stop=last, tile_position=(0, 0),
                                         skip_group_check=True)
                        nc.tensor.matmul(av[DK:P, :], lhsT=vtile[:, DK:P],
                                         rhs=ep[:, 1, :], start=first,
                                         stop=last, tile_position=(0, 64),
                                         skip_group_check=True)
                        nc.tensor.matmul(z[0:1, :], lhsT=ones_bf_sb[:],
                                         rhs=ep[:, 0, :], start=first,
                                         stop=last, tile_position=(0, 0),
                                         skip_group_check=True)
                        nc.tensor.matmul(z[32:33, :], lhsT=ones_bf_sb[:],
                                         rhs=ep[:, 1, :], start=first,
                                         stop=last, tile_position=(0, 32),
                                         skip_group_check=True)

                def spill(pair):
                    av, z = avz[pair]
                    asp = small.tile([P, T], F32, tag=f"asp{pair}", bufs=1,
                                     name=f"asp{pair}")
                    nc.vector.tensor_copy(asp[:], av[:])
                    zsp = small.tile([33, T], F32, tag=f"zsp{pair}", bufs=1,
                                     name=f"zsp{pair}")
                    nc.vector.tensor_copy(zsp[:], z[0:33, :])
                    spl[pair] = (asp, zsp)

                def finalize(pair, spilled):
                    av, z = avz[pair]
                    if spilled:
                        asp, zsp = spl[pair]
                        zt = small.tile([33, T], F32, tag="zt", bufs=1,
                                        name="zt")
                        nc.vector.tensor_tensor(zt[:], z[0:33, :], zsp[:],
                                                OP.add)
                        zin = zt[:]
                    else:
                        zin = z[0:33, :]
                    lnz = small.tile([33, T], F32, tag="lnz", name="lnz")
                    nc.scalar.activation(lnz[:], zin, AF.Ln, bias=EPS)
                    reca = small.tile([1, T], F32, tag="reca", name="reca")
                    nc.scalar.activation(reca[:], lnz[0:1, :], AF.Exp,
                                         scale=-1.0)
                    recb = small.tile([1, T], F32, tag="recb", name="recb")
                    nc.scalar.activation(recb[:], lnz[32:33, :], AF.Exp,
                                         scale=-1.0)
                    bca = small.tile([P, T], F32, tag="bca", bufs=1,
                                     name="bca")
                    nc.gpsimd.partition_broadcast(bca[:], reca[:])
                    bcb = small.tile([P, T], F32, tag="bcb", bufs=1,
                                     name="bcb")
                    nc.gpsimd.partition_broadcast(bcb[:], recb[:])
                    if spilled:
                        asp, _ = spl[pair]
                        avt = small.tile([P, T], F32, tag="avt", bufs=1,
                                         name="avt")
                        nc.vector.tensor_tensor(avt[:], av[:], asp[:], OP.add)
                        src = avt
                    else:
                        src = av
                    nc.vector.tensor_tensor(attnT[0:DK, pair, :], src[0:DK, :],
                                            bca[0:DK, :], OP.mult)
                    nc.vector.tensor_tensor(attnT[DK:P, pair, :], src[DK:P, :],
                                            bcb[DK:P, :], OP.mult)

                # single pair in flight (av/z = 2 PSUM banks), 3-deep score
                # pipeline: part A for all pairs (spill av/z to SBUF), then
                # part B (restore-combine).  AG1 hides under part A.
                load_part(0, 0)
                load_part(1, 0)
                attn_part(0, 0, True, True)
                spill(0)
                attn_part(1, 0, True, True)
                spill(1)
                q_chunk(2)
                q_chunk(3)
                load_part(2, 0)
                load_part(2, 1)
                load_part(3, 0)
                load_part(3, 1)
                attn_part(2, 0, True, False)
                attn_part(2, 1, False, True)
                finalize(2, False)
                attn_part(3, 0, True, False)
                attn_part(3, 1, False, True)
                finalize(3, False)
                load_part(0, 1)
                load_part(1, 1)
                attn_part(0, 1, True, True)
                finalize(0, True)
                attn_part(1, 1, True, True)
                finalize(1, True)

                # ---- output projection + residual (+ LN2 stats) ----
                ln2a_ctx = ln_begin(0)
                ln2b_ctx = ln_begin(1)
                for m in range(NJ):
                    ps = ps_sc.tile([P, T], F32, tag="s", name="omm")
                    for ji, j in enumerate((2, 3, 0, 1)):
                        nc.tensor.matmul(
                            ps[:], lhsT=w_sb["wo"][:, j, m * P:(m + 1) * P],
                            rhs=attnT[:, j, :],
                            start=(ji == 0), stop=(ji == NJ - 1),
                            tile_position=(0, 0), skip_group_check=True)
                    nc.vector.tensor_add(xTh[0][:, m, :], xTh[0][:, m, :],
                                         ps[:, 0:HALF])
                    nc.vector.tensor_add(xTh[1][:, m, :], xTh[1][:, m, :],
                                         ps[:, HALF:T])
                    ln_stats(ln2a_ctx, xTh[0], m)
                    ln_stats(ln2b_ctx, xTh[1], m)

                # ---- LN2 + FFN (A now, B deferred); the f2 loop emits the
                # next LN's stats right after each residual-add chunk ----
                def ffn_half(w_ref, gT, h, ln_ctx):
                    h1T = work.tile([P, NM, HALF], BF16, tag=f"h1{h}",
                                    name="h1T")
                    for m in range(NM):
                        ps = ps_sc.tile([P, HALF], F32, tag="s", name="f1mm")
                        for j in range(NJ):
                            nc.tensor.matmul(
                                ps[:], lhsT=w_ref["w1"][:, j, m * P:(m + 1) * P],
                                rhs=gT[:, j, :],
                                start=(j == 0), stop=(j == NJ - 1),
                                tile_position=(0, 0))
                        if use_b1:
                            nc.vector.tensor_scalar(h1T[:, m, :], ps[:],
                                                    w_ref["b1"][:, m:m + 1],
                                                    0.0, OP.add, OP.max)
                        else:
                            nc.vector.tensor_scalar(h1T[:, m, :], ps[:], 0.0,
                                                    None, OP.max)
                    for m in range(NJ):
                        ps = ps_sc.tile([P, HALF], F32, tag="s", name="f2mm")
                        for j in range(NM):
                            nc.tensor.matmul(
                                ps[:], lhsT=w_ref["w2"][:, j, m * P:(m + 1) * P],
                                rhs=h1T[:, j, :],
                                start=(j == 0), stop=(j == NM - 1),
                                tile_position=(0, 0), skip_group_check=True)
                        xsl = xTh[h][:, m, :]
                        if use_b2:
                            tmp = small.tile([P, HALF], F32, tag="b2tmp",
                                             name="b2tmp")
                            nc.vector.tensor_scalar(tmp[:], ps[:],
                                                    w_ref["b2"][:, m:m + 1],
                                                    None, OP.add)
                            nc.vector.tensor_add(xsl, xsl, tmp[:])
                        else:
                            nc.vector.tensor_add(xsl, xsl, ps[:])
                        ln_stats(ln_ctx, xTh[h], m)

                gTA = work.tile([P, NJ, HALF], BF16, tag="gTA", name="gTA")
                ln_finish(ln2a_ctx, gTA, xTh[0], 0, 2 * l + 1, use_ln2)
                ln1a_ctx = ln_begin(0)
                ffn_half(w_sb, gTA, 0, ln1a_ctx)
                gTB = work.tile([P, NJ, HALF], BF16, tag="gTB", name="gTB")
                ln_finish(ln2b_ctx, gTB, xTh[1], 1, 2 * l + 1, use_ln2)

                def make_tail(w_ref, gT):
                    def tail():
                        ctx = ln_begin(1)
                        ffn_half(w_ref, gT, 1, ctx)
                        return ctx
                    return tail

                prev_tail[0] = make_tail(w_sb, gTB)
                if l + 1 < L:
                    w_sb = w_next

            # ---------------- final: FFN(5)_B + LN + output ----------------
            ln1b_ctx = prev_tail[0]()
            out_sb = work.tile([P, NJ, D], F32, tag="osb", name="out_sb")
            for h, ctx in ((0, ln1a_ctx), (1, ln1b_ctx)):
                outh = work.tile([P, NJ, HALF], F32, tag="outh", name="outh")
                ln_finish(ctx, outh, xTh[h], h, 2 * L, use_fn)
                for dj in range(NJ):
                    for tj in range(HC):
                        pst = ps_sc.tile([P, P], F32, tag="s", name="otr")
                        nc.tensor.transpose(
                            pst[:], outh[:, dj, tj * P:(tj + 1) * P],
                            ident_sb[:])
                        nc.vector.tensor_copy(
                            out_sb[:, HC * h + tj, dj * P:(dj + 1) * P],
                            pst[:])
            nc.sync.dma_start(y.rearrange("(j p) d -> p j d", p=P), out_sb[:])
        import sys, time
        print(f"[build] body traced {time.time():.0f}", file=sys.stderr,
              flush=True)

    print(f"[build] tile scheduled {time.time():.0f}", file=sys.stderr,
          flush=True)
    nc.compile()
    print(f"[build] bacc compiled {time.time():.0f}", file=sys.stderr,
          flush=True)
    return nc


# ---------------------------------------------------------------------------
_CACHE = {}


def _get_kernel(flags):
    if flags not in _CACHE:
        _CACHE[flags] = build_kernel(*flags)
    return _CACHE[flags]


def _chunkP(a):
    """[..., n*P, m] -> [..., P, n, m] with dim = n_idx*P + p."""
    a = np.asarray(a)
    *lead, npm, m = a.shape
    n = npm // P
    return np.ascontiguousarray(a.reshape(*lead, n, P, m).swapaxes(-3, -2))


def kernel(**inputs):
    src = np.asarray(inputs["src"]).astype(np.int64)
    emb = np.asarray(inputs["emb"], np.float32)
    pe = np.asarray(inputs["pe"], np.float32)
    W = {k: np.asarray(inputs[k], np.float32)
         for k in ("WQ", "WK", "WV", "WO", "W1", "W2", "b1", "b2",
                   "ln1_s", "ln1_b", "ln2_s", "ln2_b", "fn_s", "fn_b")}

    use_b1 = bool(np.any(W["b1"] != 0.0))
    use_b2 = bool(np.any(W["b2"] != 0.0))
    use_ln1 = bool(np.any(W["ln1_s"] != 1.0) or np.any(W["ln1_b"] != 0.0))
    use_ln2 = bool(np.any(W["ln2_s"] != 1.0) or np.any(W["ln2_b"] != 0.0))
    use_fn = bool(np.any(W["fn_s"] != 1.0) or np.any(W["fn_b"] != 0.0))
    nc = _get_kernel((use_b1, use_b2, use_ln1, use_ln2, use_fn))

    def perD(a):  # [L, D] -> [L, P, NJ] (d = j*P + p)
        a = np.asarray(a, np.float32)
        return np.ascontiguousarray(a.reshape(-1, NJ, P).swapaxes(-2, -1))

    lnsb = np.zeros((P, 2 * L + 1, 2, NJ), np.float32)
    for l in range(L):
        lnsb[:, 2 * l, 0] = perD(W["ln1_s"])[l]
        lnsb[:, 2 * l, 1] = perD(W["ln1_b"])[l]
        lnsb[:, 2 * l + 1, 0] = perD(W["ln2_s"])[l]
        lnsb[:, 2 * l + 1, 1] = perD(W["ln2_b"])[l]
    lnsb[:, 2 * L, 0] = perD(W["fn_s"][None])[0]
    lnsb[:, 2 * L, 1] = perD(W["fn_b"][None])[0]

    shared = {
        "emb": emb,
        "wq": _chunkP(W["WQ"]).astype(ml_dtypes.bfloat16),
        "wk": _chunkP(W["WK"]).astype(ml_dtypes.bfloat16),
        "wv": _chunkP(W["WV"]).astype(ml_dtypes.bfloat16),
        "wo": _chunkP(W["WO"]).astype(ml_dtypes.bfloat16),
        "w1": _chunkP(W["W1"]).astype(ml_dtypes.bfloat16),
        "w2": _chunkP(W["W2"]).astype(ml_dtypes.bfloat16),
        "b1t": np.ascontiguousarray(W["b1"].reshape(L, NM, P).swapaxes(1, 2)),
        "b2t": np.ascontiguousarray(W["b2"].reshape(L, NJ, P).swapaxes(1, 2)),
        "lnsb": lnsb,
        "ones_bf": np.ones((P, 1), ml_dtypes.bfloat16),
        "oneD": np.full((P, 1), 1.0 / D, np.float32),
        "ones_row": np.ones((33, P), np.float32),
        "ident": np.eye(P, dtype=np.float32),
    }

    in_maps = []
    for c in range(NCORES):
        b, sc = c // GROUP, c % GROUP
        ids = src[b, sc * T:(sc + 1) * T].astype(np.int32)
        peslice = pe[sc * T:(sc + 1) * T, :]                   # [T, D]
        pett = np.ascontiguousarray(
            peslice.T.reshape(NJ, P, T).swapaxes(0, 1))        # [P, NJ, T]
        maskn = np.where(src[b] == PAD, NEG, 0.0).astype(np.float32)
        # key order: part-major. gc = part*8 + r*2 + s
        #   <-> orig token chunk tc = part*2 + s of rank r
        arr = maskn.reshape(GROUP, NJ, P)                      # [r, tc, p]
        kmask_new = np.empty((P, NC), np.float32)
        for part in range(2):
            for r in range(GROUP):
                for s_ in range(HC):
                    kmask_new[:, part * 8 + r * HC + s_] = arr[r, part * HC + s_]
        m = dict(shared)
        m["tok"] = np.ascontiguousarray(ids.reshape(NJ, P).T)
        m["pet"] = pett.astype(np.float32)
        m["kmaskn"] = kmask_new
        in_maps.append(m)

    res = run_bass_kernel_spmd(nc, in_maps, core_ids=list(range(NCORES)))
    out = np.empty((B, S, D), np.float32)
    for c in range(NCORES):
        b, sc = c // GROUP, c % GROUP
        out[b, sc * T:(sc + 1) * T, :] = res.results[c]["y"]
    return out
